# revision 1
# baseline (speedup 1.0000x reference)
"""ArcFace (AngularPenaltySMLoss) over [32768, 8192] f32, distributed over
8 TRN2 NeuronCores, data-parallel on the batch dim.

Per core: shard [4096, 8192]. For each 128-row tile:
  - DMA tile to SBUF (sync-engine HWDGE ring; one ring keeps tile
    completions sequential so the double-buffer never stalls)
  - ScalarE: exp(S*x) with fused free-dim accumulation -> row exp-sums
  - VectorE: scalar_tensor_tensor (iota == label) * x with fused free-dim
    accumulation -> gathers target = x[row, label] (one nonzero per row)
Epilogue (two batches; the first hides inside the loop):
  numerator = S*(t*cos(M) - sin(M)*sqrt(1 - t^2))   # = S*cos(acos(t)+M)
  with sqrt(y) computed as exp(0.5*ln(y)) so the only ACT table set used
  anywhere is natural_log_exp (zero mid-kernel table switches).
  L = numerator - log(exp(numerator) + rowsum - exp(S*t))
  partial = sum(L) per core -> [128,1]; host sums, loss = -total/N.
"""

import numpy as np

from concourse import bacc, hw_specs, mybir, tile
from concourse.bass_utils import run_bass_kernel_spmd

# The act-table placement pass picks the FIRST set containing each
# activation function, so an Exp/Ln mix thrashes between exp_and_others and
# natural_log (8 table loads here, 3 on the critical tail). Present a view
# of the tables with Exp/Ln stripped from every set except the combined
# natural_log_exp_and_others so both resolve to one set (one load total).
# Only membership changes; set order/ids still match act_info.json.
_ORIG_GET_TABLES = hw_specs.get_activation_tables
_COMBINED_SET = "natural_log_exp_and_others"


def _exp_ln_combined_tables(arch):
    tabs = _ORIG_GET_TABLES(arch)
    AF = mybir.ActivationFunctionType
    if _COMBINED_SET not in tabs:
        return tabs
    return {
        name: (fns - {AF.Exp, AF.Ln} if name != _COMBINED_SET else fns)
        for name, fns in tabs.items()
    }



N, C = 32768, 8192
N_CORES = 8
N_SHARD = N // N_CORES      # 4096 rows per core
P = 128                     # SBUF partitions
N_TILES = N_SHARD // P      # 32 tiles per core
S = 32.0
M = 0.5
EPS = 1e-7

_F32 = mybir.dt.float32


def build(n_shard=N_SHARD, c=C, dual_ring=False, psum_et=True, gp_cols=0):
    # gp_cols>0 (offloading part of the gather to GpSimd's
    # scalar_tensor_tensor) fails to compile in this backend — keep 0.
    # dual_ring=True (alternating x-tile DMAs across the SP and ACT HWDGE
    # rings) measured consistently ~40us SLOWER in interleaved A/B: the two
    # rings' transfers share the 16 SDMA engines, so tile k's completion is
    # delayed by tile k+1's concurrent transfer, stalling the double-buffer.
    prev_tables = bacc.get_activation_tables
    bacc.get_activation_tables = _exp_ln_combined_tables
    try:
        return _build(n_shard, c, dual_ring, psum_et, gp_cols)
    finally:
        bacc.get_activation_tables = prev_tables


def _build(n_shard, c, dual_ring, psum_et, gp_cols):
    n_tiles = n_shard // P
    nc = bacc.Bacc(None, target_bir_lowering=False)

    x_ext = nc.declare_dram_parameter("cls_score", [n_shard, c], _F32, isOutput=False)
    lab_ext = nc.declare_dram_parameter("labels_t", [P, n_tiles], _F32, isOutput=False)
    out_ext = nc.declare_dram_parameter("out", [P, 1], _F32, isOutput=True)

    AF = mybir.ActivationFunctionType
    OP = mybir.AluOpType
    AX = mybir.AxisListType

    split = n_tiles - 1 if n_tiles > 1 else 1

    # et (the exp output) is write-only scratch. With psum_et it goes to
    # PSUM (ScalarE's faster port) in two 4096-wide chunks (PSUM free-dim
    # cap). The gather also runs in two half-width chunks (A/B accumulators
    # folded in the epilogue), shrinking its scratch to half a tile; the
    # freed SBUF deepens the x stream to 4 buffers.
    x_bufs = 4 if psum_et else 2
    half = c // 2 if c > 1 else 1

    with tile.TileContext(nc) as tc:
        with (
            tc.tile_pool(name="xp", bufs=x_bufs) as xp,
            tc.tile_pool(name="ep", bufs=1,
                         space="PSUM" if psum_et else "SBUF") as ep,
            tc.tile_pool(name="mp", bufs=1) as mp,
            tc.tile_pool(name="st", bufs=1) as st,
        ):
            lab = st.tile([P, n_tiles], _F32)
            nc.scalar.dma_start(out=lab[:], in_=lab_ext[:])
            iota = st.tile([P, c], _F32)  # each row = [0..c-1]
            nc.gpsimd.iota(iota[:], pattern=[[1, c]], base=0,
                           channel_multiplier=0,
                           allow_small_or_imprecise_dtypes=True)

            sumexp = st.tile([P, n_tiles], _F32)
            sumexpA = st.tile([P, n_tiles], _F32)  # first-half chunk sums
            sumexpB = st.tile([P, n_tiles], _F32)  # second-half chunk sums
            tvals = st.tile([P, n_tiles], _F32)
            # gather accumulators per column range; the range without the
            # label sums to exactly 0, so tvals = A + B
            tvalsA = st.tile([P, n_tiles], _F32)
            tvalsB = st.tile([P, n_tiles], _F32)

            # epilogue scratch, written in column batches
            tclip = st.tile([P, n_tiles], _F32)
            tsq = st.tile([P, n_tiles], _F32)
            om = st.tile([P, n_tiles], _F32)
            lnom = st.tile([P, n_tiles], _F32)
            r = st.tile([P, n_tiles], _F32)
            b_t = st.tile([P, n_tiles], _F32)
            num = st.tile([P, n_tiles], _F32)
            e_num = st.tile([P, n_tiles], _F32)
            e_st = st.tile([P, n_tiles], _F32)
            excl = st.tile([P, n_tiles], _F32)
            denom = st.tile([P, n_tiles], _F32)
            logd = st.tile([P, n_tiles], _F32)
            ell = st.tile([P, n_tiles], _F32)

            def epilogue(sl):
                # all [P, width] ops; only Exp/Ln on ACT (one table set)
                if psum_et:
                    nc.vector.tensor_tensor(
                        sumexp[:, sl], sumexpA[:, sl], sumexpB[:, sl], OP.add)
                nc.vector.tensor_tensor(
                    tvals[:, sl], tvalsA[:, sl], tvalsB[:, sl], OP.add)
                nc.vector.tensor_scalar(
                    tclip[:, sl], tvals[:, sl], -1.0 + EPS, 1.0 - EPS,
                    OP.max, OP.min)
                nc.vector.tensor_tensor(tsq[:, sl], tclip[:, sl], tclip[:, sl],
                                        OP.mult)
                nc.vector.tensor_scalar(om[:, sl], tsq[:, sl], -1.0, 1.0,
                                        OP.mult, OP.add)  # 1 - t^2
                nc.scalar.activation(out=lnom[:, sl], in_=om[:, sl], func=AF.Ln)
                nc.scalar.activation(out=r[:, sl], in_=lnom[:, sl], func=AF.Exp,
                                     scale=0.5)  # sqrt(1-t^2)
                nc.vector.tensor_scalar_mul(b_t[:, sl], r[:, sl],
                                            S * float(np.sin(M)))
                nc.vector.scalar_tensor_tensor(
                    num[:, sl], tclip[:, sl], S * float(np.cos(M)), b_t[:, sl],
                    OP.mult, OP.subtract)
                nc.scalar.activation(out=e_num[:, sl], in_=num[:, sl], func=AF.Exp)
                nc.scalar.activation(out=e_st[:, sl], in_=tvals[:, sl],
                                     func=AF.Exp, scale=S)
                nc.vector.scalar_tensor_tensor(
                    excl[:, sl], e_st[:, sl], -1.0, sumexp[:, sl],
                    OP.mult, OP.add)  # sumexp - exp(S t)
                nc.vector.tensor_tensor(denom[:, sl], excl[:, sl], e_num[:, sl],
                                        OP.add)
                nc.scalar.activation(out=logd[:, sl], in_=denom[:, sl], func=AF.Ln)
                nc.vector.tensor_tensor(ell[:, sl], num[:, sl], logd[:, sl],
                                        OP.subtract)

            halves = (slice(0, half), slice(half, c))
            acc_cols = (sumexpA, sumexpB)

            def do_exp(xt, k, col_off=0):
                # exp(S*x) + row-sum; two PSUM-sized chunks when psum_et
                if psum_et:
                    for h, cs in enumerate(halves):
                        et = ep.tile([P, half], _F32)
                        nc.scalar.activation(
                            out=et[:], in_=xt[:, cs], func=AF.Exp, scale=S,
                            accum_out=acc_cols[h][:, k:k + 1],
                        )
                else:
                    et = ep.tile([P, c], _F32)
                    nc.scalar.activation(
                        out=et[:], in_=xt[:], func=AF.Exp, scale=S,
                        accum_out=sumexp[:, k:k + 1],
                    )

            for k in range(n_tiles):
                ring = nc.scalar if (dual_ring and k % 2) else nc.sync
                if k < n_tiles - 1 or n_tiles == 1:
                    xt = xp.tile([P, c], _F32)
                    ring.dma_start(out=xt[:], in_=x_ext[k * P:(k + 1) * P, :])
                    do_exp(xt, k)
                    # (iota == label) * x ; accum -> x[row, label], in two
                    # half-width chunks so the scratch is half a tile
                    t_acc = (tvalsA, tvalsB)
                    for h, cs in enumerate(halves):
                        mt = mp.tile([P, half], _F32)
                        nc.vector.scalar_tensor_tensor(
                            mt[:], iota[:, cs], lab[:, k:k + 1], xt[:, cs],
                            OP.is_equal, OP.mult,
                            accum_out=t_acc[h][:, k:k + 1],
                        )
                else:
                    # last tile in column halves: only ~half a tile of
                    # compute remains exposed after the final transfer
                    t_acc = (tvalsA, tvalsB)
                    for h, cs in enumerate(halves):
                        xt = xp.tile([P, half], _F32)
                        (nc.scalar if (dual_ring and h % 2) else nc.sync).dma_start(
                            out=xt[:], in_=x_ext[k * P:(k + 1) * P, cs])
                        et = ep.tile([P, half], _F32)
                        nc.scalar.activation(
                            out=et[:], in_=xt[:], func=AF.Exp, scale=S,
                            accum_out=acc_cols[h][:, k:k + 1],
                        )
                        mt = mp.tile([P, half], _F32)
                        # iota slice keeps global column indices
                        nc.vector.scalar_tensor_tensor(
                            mt[:], iota[:, cs], lab[:, k:k + 1], xt[:],
                            OP.is_equal, OP.mult,
                            accum_out=t_acc[h][:, k:k + 1],
                        )
                    if not psum_et:
                        nc.vector.tensor_tensor(
                            sumexp[:, k:k + 1], sumexpA[:, k:k + 1],
                            sumexpB[:, k:k + 1], OP.add)
                if k == split - 1 and n_tiles > 1:
                    epilogue(slice(0, split))

            epilogue(slice(split, n_tiles) if n_tiles > 1 else slice(0, n_tiles))

            lrow = st.tile([P, 1], _F32)
            nc.vector.tensor_reduce(lrow[:], ell[:], axis=AX.X, op=OP.add)
            nc.scalar.dma_start(out=out_ext[:], in_=lrow[:])

    nc.finalize()
    return nc


_NC_CACHE = {}


def _get_nc():
    if "nc" not in _NC_CACHE:
        _NC_CACHE["nc"] = build()
    return _NC_CACHE["nc"]


def make_in_maps(cls_score, labels):
    cls_score = np.ascontiguousarray(np.asarray(cls_score, dtype=np.float32))
    labels = np.asarray(labels).astype(np.int64)
    in_maps = []
    for i in range(N_CORES):
        shard = cls_score[i * N_SHARD:(i + 1) * N_SHARD]
        lab_i = labels[i * N_SHARD:(i + 1) * N_SHARD].astype(np.float32)
        # [n_tiles, P] -> [P, n_tiles]: partition p, col k = label of row k*P+p
        lab_t = np.ascontiguousarray(lab_i.reshape(N_TILES, P).T)
        in_maps.append({"cls_score": shard, "labels_t": lab_t})
    return in_maps


def kernel(cls_score, labels):
    nc = _get_nc()
    in_maps = make_in_maps(cls_score, labels)
    res = run_bass_kernel_spmd(nc, in_maps, core_ids=list(range(N_CORES)))
    total = np.sum(
        [r["out"].astype(np.float64).sum() for r in res.results]
    )
    return np.float32(-(total / N))



# revision 11
# speedup vs baseline: 1.1057x; 1.1057x over previous
"""ArcFace (AngularPenaltySMLoss) over [32768, 8192] f32, distributed over
8 TRN2 NeuronCores, data-parallel on the batch dim.

Per core: shard [4096, 8192], reshaped host-side to [16, 128, 16384] so
each SBUF partition holds TWO consecutive rows (64KB contiguous per
partition per tile). HWDGE splits a transfer into contiguous
partition-blocks of count/16 per DMA engine (partition count must be a
multiple of 16 — a 127-row transfer degenerates to ONE engine), and the
ring-housekeeping engine E79 runs ~13% slower while busy, pacing the
stream. Two rows per partition halves the descriptor count per byte
(128 x 64KB vs 256 x 32KB per 256 rows), trimming E79's overhead.

Per [128, 16384] tile (one 8MB transfer; 16 tiles, no remainder):
  - ScalarE: exp(S*x) in four 4096-wide chunks (PSUM free-dim cap) with
    fused accumulation; chunks 0,1 are row 2p (-> col 2k), chunks 2,3
    row 2p+1 (-> col 2k+1).
  - GpSimd ap_gather on each 8192-col half: pulls x[p, lab[16*(p//16)+i]]
    into a [128,16] block (indices wrap per 16-partition group);
    VectorE scalar_tensor_tensor with a diagonal mask extracts
    x[p, lab[p]] -> tvals. Replaces the old iota==label scan that kept
    VectorE 84% busy and rate-matched with the DMA stream.
Last tile is fetched in four 4096-col transfers and its final chunk
exp'd in two 2048-wide pieces, so only ~2.5us of ScalarE work is
exposed after the final DMA byte; the epilogue for cols 0..29 hides
under the last tile's stream.
Epilogue:
  numerator = S*(t*cos(M) - sin(M)*sqrt(1 - t^2))   # = S*cos(acos(t)+M)
  with sqrt(y) = exp(0.5*ln(y)) so the only ACT table set used is
  natural_log_exp (zero mid-kernel table switches).
  L = numerator - log(exp(numerator) + rowsum - exp(S*t))
Final: GpSimd XYZWC-reduce of ell[128,32] to [1,1] and a 4-byte out-DMA
(a [128,1] out costs ~6us of trailing per-engine semaphore straggle).
"""

import numpy as np

from concourse import bacc, hw_specs, mybir, tile
from concourse.bass_utils import run_bass_kernel_spmd

# The act-table placement pass picks the FIRST set containing each
# activation function, so an Exp/Ln mix thrashes between exp_and_others and
# natural_log (8 table loads, 3 on the critical tail). Present a view
# of the tables with Exp/Ln stripped from every set except the combined
# natural_log_exp_and_others so both resolve to one set (one load total).
# Only membership changes; set order/ids still match act_info.json.
_ORIG_GET_TABLES = hw_specs.get_activation_tables
_COMBINED_SET = "natural_log_exp_and_others"


def _exp_ln_combined_tables(arch):
    tabs = _ORIG_GET_TABLES(arch)
    AF = mybir.ActivationFunctionType
    if _COMBINED_SET not in tabs:
        return tabs
    return {
        name: (fns - {AF.Exp, AF.Ln} if name != _COMBINED_SET else fns)
        for name, fns in tabs.items()
    }


N, C = 32768, 8192
N_CORES = 8
N_SHARD = N // N_CORES      # 4096 rows per core
P = 128                     # SBUF partitions
RPP = 2                     # rows per partition
TROWS = P * RPP             # 256 rows per tile
N_T = N_SHARD // TROWS      # 16 tiles per core
W = C * RPP                 # 16384 cols per tile
NCOL = N_T * RPP            # 32 per-row-state columns
S = 32.0
M = 0.5
EPS = 1e-7

_F32 = mybir.dt.float32
_I16 = mybir.dt.int16


def build(out_scalar=True, x_bufs=2):
    prev_tables = bacc.get_activation_tables
    bacc.get_activation_tables = _exp_ln_combined_tables
    try:
        return _build(out_scalar, x_bufs)
    finally:
        bacc.get_activation_tables = prev_tables


def _build(out_scalar, x_bufs):
    nc = bacc.Bacc(None, target_bir_lowering=False)

    x_ext = nc.declare_dram_parameter("cls_score", [N_T, P, W], _F32,
                                      isOutput=False)
    # gather-A index of tile k at col 4k, gather-B at col 4k+2: the
    # index operand needs a 4-byte-aligned SBUF offset
    lab_ext = nc.declare_dram_parameter("labels_t", [P, 4 * N_T], _I16,
                                        isOutput=False)
    diag_ext = nc.declare_dram_parameter("diag16", [P, 16], _F32,
                                         isOutput=False)
    out_shape = [1, 1] if out_scalar else [P, 1]
    out_ext = nc.declare_dram_parameter("out", out_shape, _F32, isOutput=True)

    AF = mybir.ActivationFunctionType
    OP = mybir.AluOpType
    AX = mybir.AxisListType

    CH = 4096               # exp chunk width (PSUM free-dim cap)

    with tile.TileContext(nc) as tc:
        with (
            tc.tile_pool(name="xp", bufs=x_bufs) as xp,
            tc.tile_pool(name="ep", bufs=1, space="PSUM") as ep,
            tc.tile_pool(name="mp", bufs=2) as mp,
            tc.tile_pool(name="st", bufs=1) as st,
        ):
            lab = st.tile([P, 4 * N_T], _I16)
            nc.scalar.dma_start(out=lab[:], in_=lab_ext[:])
            diag = st.tile([P, 16], _F32)
            nc.scalar.dma_start(out=diag[:], in_=diag_ext[:])

            sumexp = st.tile([P, NCOL], _F32)
            sumexpA = st.tile([P, NCOL], _F32)  # first-half chunk sums
            sumexpB = st.tile([P, NCOL], _F32)  # second-half chunk sums
            tailacc = st.tile([P, 2], _F32)     # last chunk's 2048-wide pair
            tvals = st.tile([P, NCOL], _F32)

            # epilogue scratch, written in column batches
            tclip = st.tile([P, NCOL], _F32)
            tsq = st.tile([P, NCOL], _F32)
            om = st.tile([P, NCOL], _F32)
            lnom = st.tile([P, NCOL], _F32)
            r = st.tile([P, NCOL], _F32)
            b_t = st.tile([P, NCOL], _F32)
            num = st.tile([P, NCOL], _F32)
            e_num = st.tile([P, NCOL], _F32)
            e_st = st.tile([P, NCOL], _F32)
            excl = st.tile([P, NCOL], _F32)
            denom = st.tile([P, NCOL], _F32)
            logd = st.tile([P, NCOL], _F32)
            ell = st.tile([P, NCOL], _F32)

            def epilogue(sl):
                # all [P, width] ops; only Exp/Ln on ACT (one table set)
                nc.vector.tensor_scalar(
                    tclip[:, sl], tvals[:, sl], -1.0 + EPS, 1.0 - EPS,
                    OP.max, OP.min)
                nc.vector.tensor_tensor(tsq[:, sl], tclip[:, sl],
                                        tclip[:, sl], OP.mult)
                nc.vector.tensor_scalar(om[:, sl], tsq[:, sl], -1.0, 1.0,
                                        OP.mult, OP.add)  # 1 - t^2
                nc.scalar.activation(out=lnom[:, sl], in_=om[:, sl],
                                     func=AF.Ln)
                nc.scalar.activation(out=r[:, sl], in_=lnom[:, sl],
                                     func=AF.Exp, scale=0.5)  # sqrt(1-t^2)
                nc.vector.tensor_scalar_mul(b_t[:, sl], r[:, sl],
                                            S * float(np.sin(M)))
                nc.vector.scalar_tensor_tensor(
                    num[:, sl], tclip[:, sl], S * float(np.cos(M)),
                    b_t[:, sl], OP.mult, OP.subtract)
                nc.scalar.activation(out=e_num[:, sl], in_=num[:, sl],
                                     func=AF.Exp)
                nc.scalar.activation(out=e_st[:, sl], in_=tvals[:, sl],
                                     func=AF.Exp, scale=S)
                nc.vector.scalar_tensor_tensor(
                    excl[:, sl], e_st[:, sl], -1.0, sumexp[:, sl],
                    OP.mult, OP.add)  # sumexp - exp(S t)
                nc.vector.tensor_tensor(denom[:, sl], excl[:, sl],
                                        e_num[:, sl], OP.add)
                nc.scalar.activation(out=logd[:, sl], in_=denom[:, sl],
                                     func=AF.Ln)
                nc.vector.tensor_tensor(ell[:, sl], num[:, sl], logd[:, sl],
                                        OP.subtract)

            def do_exp(xt, chunk_cs, acc_ap):
                et = ep.tile([P, chunk_cs.stop - chunk_cs.start], _F32)
                nc.scalar.activation(
                    out=et[:], in_=xt[:, chunk_cs], func=AF.Exp, scale=S,
                    accum_out=acc_ap,
                )

            def gather(xt, k, half_idx):
                # gather on one 8192-col half; plain labels index the half
                g = mp.tile([P, 16], _F32)
                cs = slice(half_idx * C, (half_idx + 1) * C)
                nc.gpsimd.ap_gather(g[:], xt[:, cs],
                                    lab[:, 4 * k + 2 * half_idx:
                                        4 * k + 2 * half_idx + 1],
                                    channels=P, num_elems=C, d=1, num_idxs=16)
                mt = mp.tile([P, 16], _F32)
                nc.vector.scalar_tensor_tensor(
                    mt[:], g[:], 1.0, diag[:], OP.mult, OP.mult,
                    accum_out=tvals[:, RPP * k + half_idx:
                                    RPP * k + half_idx + 1])

            acc_of = {0: sumexpA, 1: sumexpB, 2: sumexpA, 3: sumexpB}

            for k in range(N_T - 1):
                xt = xp.tile([P, W], _F32)
                nc.sync.dma_start(out=xt[:], in_=x_ext[k, :, :])
                for q in range(4):
                    col = RPP * k + q // 2
                    do_exp(xt, slice(q * CH, (q + 1) * CH),
                           acc_of[q][:, col:col + 1])
                gather(xt, k, 0)
                gather(xt, k, 1)

            # last tile: four 4096-col transfers; final chunk exp'd in two
            # 2048-wide pieces so little ScalarE work trails the last byte
            k = N_T - 1
            xt = xp.tile([P, W], _F32)
            for q in range(4):
                cs = slice(q * CH, (q + 1) * CH)
                nc.sync.dma_start(out=xt[:, cs], in_=x_ext[k, :, cs])
                col = RPP * k + q // 2
                if q < 3:
                    do_exp(xt, cs, acc_of[q][:, col:col + 1])
                else:
                    do_exp(xt, slice(3 * CH, 3 * CH + CH // 2),
                           tailacc[:, 0:1])
                    do_exp(xt, slice(3 * CH + CH // 2, 4 * CH),
                           tailacc[:, 1:2])
                if q == 1:
                    gather(xt, k, 0)
                if q == 2:
                    # cols 0..29 complete; their epilogue hides under the
                    # last tile's remaining stream
                    nc.vector.tensor_tensor(
                        sumexp[:, 0:NCOL - 2], sumexpA[:, 0:NCOL - 2],
                        sumexpB[:, 0:NCOL - 2], OP.add)
                    epilogue(slice(0, NCOL - 2))
            gather(xt, k, 1)
            nc.vector.tensor_reduce(sumexpB[:, NCOL - 1:NCOL], tailacc[:],
                                    axis=AX.X, op=OP.add)
            nc.vector.tensor_tensor(
                sumexp[:, NCOL - 2:NCOL], sumexpA[:, NCOL - 2:NCOL],
                sumexpB[:, NCOL - 2:NCOL], OP.add)
            epilogue(slice(NCOL - 2, NCOL))

            if out_scalar:
                osb = st.tile([1, 1], _F32)
                nc.gpsimd.tensor_reduce(osb[:], ell[:], axis=AX.XYZWC,
                                        op=OP.add)
                nc.sync.dma_start(out=out_ext[:], in_=osb[:])
            else:
                lrow = st.tile([P, 1], _F32)
                nc.vector.tensor_reduce(lrow[:], ell[:], axis=AX.X, op=OP.add)
                nc.sync.dma_start(out=out_ext[:], in_=lrow[:])

    nc.finalize()
    return nc


_NC_CACHE = {}


def _get_nc():
    if "nc" not in _NC_CACHE:
        _NC_CACHE["nc"] = build()
    return _NC_CACHE["nc"]


def make_in_maps(cls_score, labels):
    cls_score = np.ascontiguousarray(np.asarray(cls_score, dtype=np.float32))
    labels = np.asarray(labels).astype(np.int64)
    diag = np.zeros((P, 16), np.float32)
    diag[np.arange(P), np.arange(P) % 16] = 1.0
    in_maps = []
    for i in range(N_CORES):
        shard = cls_score[i * N_SHARD:(i + 1) * N_SHARD]
        li = labels[i * N_SHARD:(i + 1) * N_SHARD]
        # partition p of tile k holds rows k*256 + 2p, 2p+1
        x3 = shard.reshape(N_T, P, W)
        lr = li.reshape(N_T, P, RPP)          # [k, p, row-in-partition]
        lab16 = np.zeros((P, 4 * N_T), np.int16)
        lab16[:, 0::4] = lr[:, :, 0].T        # gather-A: even rows
        lab16[:, 2::4] = lr[:, :, 1].T        # gather-B: odd rows
        in_maps.append({
            "cls_score": x3,
            "labels_t": np.ascontiguousarray(lab16),
            "diag16": diag,
        })
    return in_maps


def kernel(cls_score, labels):
    nc = _get_nc()
    in_maps = make_in_maps(cls_score, labels)
    res = run_bass_kernel_spmd(nc, in_maps, core_ids=list(range(N_CORES)))
    total = np.sum(
        [r["out"].astype(np.float64).sum() for r in res.results]
    )
    return np.float32(-(total / N))
